# revision 15
# baseline (speedup 1.0000x reference)
"""Mixtral-style MoE layer on 8 Trainium2 NeuronCores (Bass/Tile).

Strategy: expert parallel. Core e owns expert e's SwiGLU MLP
(w1/w3/w2). The router (gate matmul + softmax + top-2 + renormalize)
is computed on host (0.03% of the FLOPs); tokens routed to expert e
are gathered, padded to a common capacity C, and shipped to core e in
transposed layout [H, C]. Each core then computes

    yT_e = w2 @ (silu(w1 @ xT_e) * (w3 @ xT_e)) * s_e

entirely on the tensor engine (fp16 matmuls, fp32 accumulate),
with the per-token routing weight s_e folded into the final PSUM
eviction. The host scatter-adds each expert's [C, H] result back into
the full [T, H] output (each token appears in exactly 2 experts).

Layouts are chosen so every DMA is contiguous and every matmul
contraction dim sits on SBUF partitions:
  xT   [KO, 128, C]   xT[k,p,c]   = x_g[c, 128k+p]
  w1t  [FB, 128, KO, 128] w1t[f,p,k,m] = w1[e][128f+m, 128k+p]
  w3t  same as w1t
  w2t  [HB, 128, FB, 128] w2t[h,p,f,m] = w2[e][128h+m, 128f+p]
  sb   [128, C]       routing weight broadcast across partitions
  yT   [HB, 128, C]   yT[h,p,c]   = y_e[c, 128h+p] (fp16)

Capacity C is the max expert load rounded up to 4 only (no 128
padding). Tokens are processed in 2 chunks (~C/2) so weights stream
through SBUF only twice; within a chunk, PSUM-sized pieces of <=512
columns (first piece 256 so the first matmul starts after ~1MB of x
DMA). Weight DMAs ride the scalar-engine ring, x/s/y the sync ring,
so weight loads are never queued behind token traffic. Measured on
the fixed inputs: ~1393-1398us, tensor-engine MFU ~96%, matmul issue
gaps at the theoretical N/2.4+2.5ns.
"""

import sys

sys.path.insert(0, "/opt/trn_rl_repo")

import numpy as np

import concourse.bass as bass  # noqa: F401  (bass must import before bacc)
from concourse import bacc
import concourse.mybir as mybir
import concourse.tile as tile
from concourse.bass_utils import run_bass_kernel_spmd

E = 8
TOP_K = 2
H = 2048
F = 4096
P = 128
KO = H // P   # 16  k-blocks for stage A contraction
FB = F // P   # 32  f-blocks
HB = H // P   # 16  h-blocks

N_CORES = 8

F32 = mybir.dt.float32
F16 = mybir.dt.float16

_cache = {}


def _pieces(ch):
    """Split a chunk of ch columns into near-equal PSUM-sized pieces.
    Prefer full 512s; keep every piece >=256 when ch allows so the
    ~107ns LDWEIGHTS stays hidden behind the matmul stream."""
    if ch <= 512:
        return [ch]
    n = -(-ch // 512)
    if ch - 512 * (n - 1) >= 256:
        out = [512] * (n - 1) + [ch - 512 * (n - 1)]
    else:
        base = ch // n
        out = [base + 1 if i < ch - base * n else base for i in range(n)]
    return out


def _chunk_plan(C):
    """Chunks of <=~1080 columns (SBUF h-tile limit), first chunk
    <=1024 so startup DMA is short. Returns [(c0, CH, [(cs, cw), ...])]."""
    chunks = []
    left = C
    while left > 0:
        c = min(1024 if not chunks else 1080, left)
        # avoid a tiny tail chunk: rebalance last two
        if 0 < left - c < 256:
            c = (left + 1) // 2
        chunks.append(c)
        left -= c
    plans = []
    off = 0
    for ci, c in enumerate(chunks):
        if ci == 0 and c > 512:
            # ascending piece widths: x DMA (~16.4ns/col) is slower than
            # f=0 compute (~13.3ns/col), so later pieces get more slack
            rest = c - 256
            n = -(-rest // 512)
            if n == 2 and rest <= 832:
                widths = [256, (rest * 2) // 5, rest - (rest * 2) // 5]
            else:
                base = rest // n
                widths = [256] + [
                    base + 1 if i < rest - base * n else base
                    for i in range(n)
                ]
        else:
            widths = _pieces(c)
        subs = []
        cs = 0
        for cw in widths:
            subs.append((cs, cw))
            cs += cw
        plans.append((off, c, subs))
        off += c
    return plans


def _build(C):
    """Build + schedule the Bass module for capacity C. Returns nc."""
    nc = bacc.Bacc(None, target_bir_lowering=False)

    xT = nc.dram_tensor("xT", [KO, P, C], F16, kind="ExternalInput")
    w1t = nc.dram_tensor("w1t", [FB, P, KO, P], F16, kind="ExternalInput")
    w3t = nc.dram_tensor("w3t", [FB, P, KO, P], F16, kind="ExternalInput")
    w2t = nc.dram_tensor("w2t", [HB, P, FB, P], F16, kind="ExternalInput")
    sb = nc.dram_tensor("sb", [P, C], F32, kind="ExternalInput")
    yT = nc.dram_tensor("yT", [HB, P, C], F16, kind="ExternalOutput")

    plans = _chunk_plan(C)
    CHMAX = max(p[1] for p in plans)

    with tile.TileContext(nc) as tc:
        with (
            tc.tile_pool(name="xp", bufs=3) as xp,
            tc.tile_pool(name="hp", bufs=1) as hp,
            tc.tile_pool(name="wa", bufs=2) as wa,
            tc.tile_pool(name="wb", bufs=2) as wb,
            tc.tile_pool(name="tmp", bufs=3) as tmp,
            tc.tile_pool(name="yo", bufs=3) as yo,
            tc.tile_pool(name="cst", bufs=1) as cst,
            tc.tile_pool(name="ps", bufs=8, space="PSUM") as ps,
        ):
            s_tile = None
            for ci, (c0, CH, subs) in enumerate(plans):
                # per-piece x tiles (first matmul starts after ~2MB DMA)
                x_tiles = []
                for cs, cw in subs:
                    x_t = xp.tile([P, KO, 512], F16, tag="x", name="x_t")[:, :, :cw]
                    # split across parallel DMA queues so the first
                    # matmul isn't gated on one serial transfer; the very
                    # first piece gets the finest split (8 ways)
                    step = 2 if (ci == 0 and cs == 0) else 4
                    for g in range(0, KO, step):
                        nc.sync.dma_start(
                            x_t[:, g : g + step, :],
                            xT[
                                g : g + step, :, c0 + cs : c0 + cs + cw
                            ].rearrange("k p c -> p k c"),
                        )
                    x_tiles.append(x_t)

                h_tile = hp.tile([P, FB, CHMAX], F16, tag="h", name="h_tile")[
                    :, :, :CH
                ]

                # ---- stage A: h = silu(w1 @ x) * (w3 @ x) ----
                for f in range(FB):
                    w1_tile = wa.tile([P, KO, P], F16, tag="w1")
                    nc.scalar.dma_start(w1_tile[:], w1t[f])
                    w3_tile = wa.tile([P, KO, P], F16, tag="w3")
                    nc.scalar.dma_start(w3_tile[:], w3t[f])
                    for (cs, cw), x_t in zip(subs, x_tiles):
                        pg = ps.tile([P, 512], F32, tag="mm", name="mm")[:, :cw]
                        pu = ps.tile([P, 512], F32, tag="mm", name="mm")[:, :cw]
                        for k in range(KO):
                            nc.tensor.matmul(
                                pg[:],
                                w1_tile[:, k, :],
                                x_t[:, k, :],
                                start=(k == 0),
                                stop=(k == KO - 1),
                            )
                        for k in range(KO):
                            nc.tensor.matmul(
                                pu[:],
                                w3_tile[:, k, :],
                                x_t[:, k, :],
                                start=(k == 0),
                                stop=(k == KO - 1),
                            )
                        sg = tmp.tile([P, 512], F32, tag="sg", name="sg")[:, :cw]
                        nc.scalar.activation(
                            sg[:], pg[:], mybir.ActivationFunctionType.Silu
                        )
                        nc.vector.tensor_tensor(
                            h_tile[:, f, cs : cs + cw],
                            sg[:],
                            pu[:],
                            mybir.AluOpType.mult,
                        )

                if s_tile is None:
                    # deferred: keeps the 1MB sb transfer out of the
                    # startup window; only stage B reads it
                    s_tile = cst.tile([P, C], F32, tag="s")
                    nc.sync.dma_start(s_tile[:], sb[:, :])

                # ---- stage B: yT = (w2 @ h) * s ----
                for hb in range(HB):
                    w2_tile = wb.tile([P, FB, P], F16, tag="w2")
                    nc.scalar.dma_start(w2_tile[:], w2t[hb])
                    y_hb = yo.tile([P, CHMAX], F16, tag="y", name="y_hb")[:, :CH]
                    for cs, cw in subs:
                        py = ps.tile([P, 512], F32, tag="mm", name="mm")[:, :cw]
                        for f in range(FB):
                            nc.tensor.matmul(
                                py[:],
                                w2_tile[:, f, :],
                                h_tile[:, f, cs : cs + cw],
                                start=(f == 0),
                                stop=(f == FB - 1),
                            )
                        nc.vector.tensor_tensor(
                            y_hb[:, cs : cs + cw],
                            py[:],
                            s_tile[:, c0 + cs : c0 + cs + cw],
                            mybir.AluOpType.mult,
                        )
                        # per-piece writeback: the tail DMA after the last
                        # matmul is one piece, not the whole row
                        nc.sync.dma_start(
                            yT[hb, :, c0 + cs : c0 + cs + cw],
                            y_hb[:, cs : cs + cw],
                        )

    nc.compile()
    return nc


def kernel(hidden_states, gate_w, w1, w3, w2):
    x = np.ascontiguousarray(hidden_states, dtype=np.float32)
    gate_w = np.asarray(gate_w, dtype=np.float32)
    T = x.shape[0]

    # ---- host router (0.03% of FLOPs) ----
    # Run the router with the exact jax ops the reference uses so the
    # top-2 selection (which has tokens with ~1e-6 tie margins) matches
    # the reference bit-for-bit.
    import jax
    import jax.numpy as jnp

    router_logits = jnp.asarray(x) @ jnp.asarray(gate_w).T   # [T, E]
    probs = jax.nn.softmax(router_logits, axis=-1)
    topk_w, topk_ids = jax.lax.top_k(probs, TOP_K)
    topk_w = topk_w / jnp.sum(topk_w, axis=-1, keepdims=True)
    top2 = np.asarray(topk_ids)                              # [T, 2]
    tw = np.asarray(topk_w, dtype=np.float32)                # [T, 2]

    idx_e = []
    s_e = []
    for e in range(E):
        tok, slot = np.nonzero(top2 == e)
        idx_e.append(tok.astype(np.int64))
        s_e.append(tw[tok, slot].astype(np.float32))
    loads = [len(ix) for ix in idx_e]
    Cmax = max(loads)
    C = 4 * (-(-Cmax // 4))

    if C not in _cache:
        _cache[C] = _build(C)
    nc = _cache[C]

    in_maps = []
    for e in range(E):
        ix = idx_e[e]
        xg = np.zeros((C, H), dtype=np.float32)
        xg[: len(ix)] = x[ix]
        s = np.zeros((C,), dtype=np.float32)
        s[: len(ix)] = s_e[e]
        in_maps.append(
            {
                "xT": np.ascontiguousarray(
                    xg.T.reshape(KO, P, C).astype(np.float16)
                ),
                "w1t": np.ascontiguousarray(
                    np.asarray(w1[e], np.float32)
                    .reshape(FB, P, KO, P)
                    .transpose(0, 3, 2, 1)
                    .astype(np.float16)
                ),
                "w3t": np.ascontiguousarray(
                    np.asarray(w3[e], np.float32)
                    .reshape(FB, P, KO, P)
                    .transpose(0, 3, 2, 1)
                    .astype(np.float16)
                ),
                "w2t": np.ascontiguousarray(
                    np.asarray(w2[e], np.float32)
                    .reshape(HB, P, FB, P)
                    .transpose(0, 3, 2, 1)
                    .astype(np.float16)
                ),
                "sb": np.ascontiguousarray(np.broadcast_to(s[None, :], (P, C))),
            }
        )

    res = run_bass_kernel_spmd(nc, in_maps, core_ids=list(range(N_CORES)))

    out = np.zeros((T, H), dtype=np.float32)
    for e in range(E):
        ix = idx_e[e]
        yTr = res.results[e]["yT"]                      # [HB, P, C] fp16
        y = yTr.reshape(H, C).T.astype(np.float32)       # [C, H]
        # within one expert every token index is unique (top-2 experts
        # of a token are distinct), so fancy-index += is safe
        out[ix] += y[: len(ix)]
    return out


# revision 16
# speedup vs baseline: 1.0057x; 1.0057x over previous
"""Mixtral-style MoE layer on 8 Trainium2 NeuronCores (Bass/Tile).

Strategy: expert parallel. Core e owns expert e's SwiGLU MLP
(w1/w3/w2). The router (gate matmul + softmax + top-2 + renormalize)
is computed on host (0.03% of the FLOPs); tokens routed to expert e
are gathered, padded to a common capacity C, and shipped to core e in
transposed layout [H, C]. Each core then computes

    yT_e = w2 @ (silu(w1 @ xT_e) * (w3 @ xT_e)) * s_e

entirely on the tensor engine (fp16 matmuls, fp32 accumulate),
with the per-token routing weight s_e folded into the final PSUM
eviction. The host scatter-adds each expert's [C, H] result back into
the full [T, H] output (each token appears in exactly 2 experts).

Layouts are chosen so every DMA is contiguous and every matmul
contraction dim sits on SBUF partitions:
  xT   [KO, 128, C]   xT[k,p,c]   = x_g[c, 128k+p]
  w1t  [FB, 128, KO, 128] w1t[f,p,k,m] = w1[e][128f+m, 128k+p]
  w3t  same as w1t
  w2t  [HB, 128, FB, 128] w2t[h,p,f,m] = w2[e][128h+m, 128f+p]
  sb   [128, C]       routing weight broadcast across partitions
  yT   [HB, 128, C]   yT[h,p,c]   = y_e[c, 128h+p] (fp16)

Capacity C is the max expert load rounded up to 4 only (no 128
padding). Tokens are processed in 2 chunks (~C/2) so weights stream
through SBUF only twice; within a chunk, PSUM-sized pieces of <=512
columns (first piece 256 so the first matmul starts after ~1MB of x
DMA). Weight DMAs ride the scalar-engine ring, x/s/y the sync ring,
so weight loads are never queued behind token traffic. Measured on
the fixed inputs: ~1393-1398us, tensor-engine MFU ~96%, matmul issue
gaps at the theoretical N/2.4+2.5ns.
"""

import sys

sys.path.insert(0, "/opt/trn_rl_repo")

import numpy as np

import concourse.bass as bass  # noqa: F401  (bass must import before bacc)
from concourse import bacc
import concourse.mybir as mybir
import concourse.tile as tile
from concourse.bass_utils import run_bass_kernel_spmd

E = 8
TOP_K = 2
H = 2048
F = 4096
P = 128
KO = H // P   # 16  k-blocks for stage A contraction
FB = F // P   # 32  f-blocks
HB = H // P   # 16  h-blocks

N_CORES = 8

F32 = mybir.dt.float32
F16 = mybir.dt.float16

_cache = {}


def _pieces(ch):
    """Split a chunk of ch columns into near-equal PSUM-sized pieces.
    Prefer full 512s; keep every piece >=256 when ch allows so the
    ~107ns LDWEIGHTS stays hidden behind the matmul stream."""
    if ch <= 512:
        return [ch]
    n = -(-ch // 512)
    if ch - 512 * (n - 1) >= 256:
        out = [512] * (n - 1) + [ch - 512 * (n - 1)]
    else:
        base = ch // n
        out = [base + 1 if i < ch - base * n else base for i in range(n)]
    return out


def _chunk_plan(C):
    """Chunks of <=~1080 columns (SBUF h-tile limit), first chunk
    <=1024 so startup DMA is short. Returns [(c0, CH, [(cs, cw), ...])]."""
    chunks = []
    left = C
    while left > 0:
        c = min(1024 if not chunks else 1080, left)
        # avoid a tiny tail chunk: rebalance last two
        if 0 < left - c < 256:
            c = (left + 1) // 2
        chunks.append(c)
        left -= c
    plans = []
    off = 0
    for ci, c in enumerate(chunks):
        if ci == 0 and c > 512:
            # small leading piece, then near-equal pieces: x DMA arrival
            # tracks the compute pace at startup (early ring BW ~250GB/s)
            rest = c - 256
            n = -(-rest // 512)
            base = rest // n
            widths = [256] + [
                base + 1 if i < rest - base * n else base for i in range(n)
            ]
        else:
            widths = _pieces(c)
        subs = []
        cs = 0
        for cw in widths:
            subs.append((cs, cw))
            cs += cw
        plans.append((off, c, subs))
        off += c
    return plans


def _build(C):
    """Build + schedule the Bass module for capacity C. Returns nc."""
    nc = bacc.Bacc(None, target_bir_lowering=False)

    xT = nc.dram_tensor("xT", [KO, P, C], F16, kind="ExternalInput")
    w1t = nc.dram_tensor("w1t", [FB, P, KO, P], F16, kind="ExternalInput")
    w3t = nc.dram_tensor("w3t", [FB, P, KO, P], F16, kind="ExternalInput")
    w2t = nc.dram_tensor("w2t", [HB, P, FB, P], F16, kind="ExternalInput")
    sb = nc.dram_tensor("sb", [P, C], F32, kind="ExternalInput")
    yT = nc.dram_tensor("yT", [HB, P, C], F16, kind="ExternalOutput")

    plans = _chunk_plan(C)
    CHMAX = max(p[1] for p in plans)

    with tile.TileContext(nc) as tc:
        with (
            tc.tile_pool(name="xp", bufs=3) as xp,
            tc.tile_pool(name="hp", bufs=1) as hp,
            tc.tile_pool(name="wa", bufs=2) as wa,
            tc.tile_pool(name="wb", bufs=2) as wb,
            tc.tile_pool(name="tmp", bufs=3) as tmp,
            tc.tile_pool(name="yo", bufs=3) as yo,
            tc.tile_pool(name="cst", bufs=1) as cst,
            tc.tile_pool(name="ps", bufs=8, space="PSUM") as ps,
        ):
            s_tile = None
            for ci, (c0, CH, subs) in enumerate(plans):
                # per-piece x tiles (first matmul starts after ~2MB DMA)
                x_tiles = []
                for cs, cw in subs:
                    x_t = xp.tile([P, KO, 512], F16, tag="x", name="x_t")[:, :, :cw]
                    # split across 4 DMAs (parallel queues) so the first
                    # matmul isn't gated on one ~2MB serial transfer
                    for g in range(0, KO, 4):
                        nc.sync.dma_start(
                            x_t[:, g : g + 4, :],
                            xT[
                                g : g + 4, :, c0 + cs : c0 + cs + cw
                            ].rearrange("k p c -> p k c"),
                        )
                    x_tiles.append(x_t)

                h_tile = hp.tile([P, FB, CHMAX], F16, tag="h", name="h_tile")[
                    :, :, :CH
                ]

                # ---- stage A: h = silu(w1 @ x) * (w3 @ x) ----
                for f in range(FB):
                    w1_tile = wa.tile([P, KO, P], F16, tag="w1")
                    nc.scalar.dma_start(w1_tile[:], w1t[f])
                    w3_tile = wa.tile([P, KO, P], F16, tag="w3")
                    nc.scalar.dma_start(w3_tile[:], w3t[f])
                    for (cs, cw), x_t in zip(subs, x_tiles):
                        pg = ps.tile([P, 512], F32, tag="mm", name="mm")[:, :cw]
                        pu = ps.tile([P, 512], F32, tag="mm", name="mm")[:, :cw]
                        for k in range(KO):
                            nc.tensor.matmul(
                                pg[:],
                                w1_tile[:, k, :],
                                x_t[:, k, :],
                                start=(k == 0),
                                stop=(k == KO - 1),
                            )
                        for k in range(KO):
                            nc.tensor.matmul(
                                pu[:],
                                w3_tile[:, k, :],
                                x_t[:, k, :],
                                start=(k == 0),
                                stop=(k == KO - 1),
                            )
                        sg = tmp.tile([P, 512], F32, tag="sg", name="sg")[:, :cw]
                        nc.scalar.activation(
                            sg[:], pg[:], mybir.ActivationFunctionType.Silu
                        )
                        nc.vector.tensor_tensor(
                            h_tile[:, f, cs : cs + cw],
                            sg[:],
                            pu[:],
                            mybir.AluOpType.mult,
                        )

                if s_tile is None:
                    # deferred: keeps the 1MB sb transfer out of the
                    # startup window; only stage B reads it
                    s_tile = cst.tile([P, C], F32, tag="s")
                    nc.sync.dma_start(s_tile[:], sb[:, :])

                # ---- stage B: yT = (w2 @ h) * s ----
                for hb in range(HB):
                    w2_tile = wb.tile([P, FB, P], F16, tag="w2")
                    nc.scalar.dma_start(w2_tile[:], w2t[hb])
                    y_hb = yo.tile([P, CHMAX], F16, tag="y", name="y_hb")[:, :CH]
                    for cs, cw in subs:
                        py = ps.tile([P, 512], F32, tag="mm", name="mm")[:, :cw]
                        for f in range(FB):
                            nc.tensor.matmul(
                                py[:],
                                w2_tile[:, f, :],
                                h_tile[:, f, cs : cs + cw],
                                start=(f == 0),
                                stop=(f == FB - 1),
                            )
                        nc.vector.tensor_tensor(
                            y_hb[:, cs : cs + cw],
                            py[:],
                            s_tile[:, c0 + cs : c0 + cs + cw],
                            mybir.AluOpType.mult,
                        )
                        # per-piece writeback: the tail DMA after the last
                        # matmul is one piece, not the whole row
                        nc.sync.dma_start(
                            yT[hb, :, c0 + cs : c0 + cs + cw],
                            y_hb[:, cs : cs + cw],
                        )

    nc.compile()
    return nc


def kernel(hidden_states, gate_w, w1, w3, w2):
    x = np.ascontiguousarray(hidden_states, dtype=np.float32)
    gate_w = np.asarray(gate_w, dtype=np.float32)
    T = x.shape[0]

    # ---- host router (0.03% of FLOPs) ----
    # Run the router with the exact jax ops the reference uses so the
    # top-2 selection (which has tokens with ~1e-6 tie margins) matches
    # the reference bit-for-bit.
    import jax
    import jax.numpy as jnp

    router_logits = jnp.asarray(x) @ jnp.asarray(gate_w).T   # [T, E]
    probs = jax.nn.softmax(router_logits, axis=-1)
    topk_w, topk_ids = jax.lax.top_k(probs, TOP_K)
    topk_w = topk_w / jnp.sum(topk_w, axis=-1, keepdims=True)
    top2 = np.asarray(topk_ids)                              # [T, 2]
    tw = np.asarray(topk_w, dtype=np.float32)                # [T, 2]

    idx_e = []
    s_e = []
    for e in range(E):
        tok, slot = np.nonzero(top2 == e)
        idx_e.append(tok.astype(np.int64))
        s_e.append(tw[tok, slot].astype(np.float32))
    loads = [len(ix) for ix in idx_e]
    Cmax = max(loads)
    C = 4 * (-(-Cmax // 4))

    if C not in _cache:
        _cache[C] = _build(C)
    nc = _cache[C]

    in_maps = []
    for e in range(E):
        ix = idx_e[e]
        xg = np.zeros((C, H), dtype=np.float32)
        xg[: len(ix)] = x[ix]
        s = np.zeros((C,), dtype=np.float32)
        s[: len(ix)] = s_e[e]
        in_maps.append(
            {
                "xT": np.ascontiguousarray(
                    xg.T.reshape(KO, P, C).astype(np.float16)
                ),
                "w1t": np.ascontiguousarray(
                    np.asarray(w1[e], np.float32)
                    .reshape(FB, P, KO, P)
                    .transpose(0, 3, 2, 1)
                    .astype(np.float16)
                ),
                "w3t": np.ascontiguousarray(
                    np.asarray(w3[e], np.float32)
                    .reshape(FB, P, KO, P)
                    .transpose(0, 3, 2, 1)
                    .astype(np.float16)
                ),
                "w2t": np.ascontiguousarray(
                    np.asarray(w2[e], np.float32)
                    .reshape(HB, P, FB, P)
                    .transpose(0, 3, 2, 1)
                    .astype(np.float16)
                ),
                "sb": np.ascontiguousarray(np.broadcast_to(s[None, :], (P, C))),
            }
        )

    res = run_bass_kernel_spmd(nc, in_maps, core_ids=list(range(N_CORES)))

    out = np.zeros((T, H), dtype=np.float32)
    for e in range(E):
        ix = idx_e[e]
        yTr = res.results[e]["yT"]                      # [HB, P, C] fp16
        y = yTr.reshape(H, C).T.astype(np.float32)       # [C, H]
        # within one expert every token index is unique (top-2 experts
        # of a token are distinct), so fancy-index += is safe
        out[ix] += y[: len(ix)]
    return out
